# revision 66
# baseline (speedup 1.0000x reference)
"""Two-layer GAT on 8 Trainium2 NeuronCores (Bass/Tile).

Host (numpy): append self-loops, degree-sort nodes (desc), pad node count to
VPAD (multiple of 8*128) and assign sorted nodes round-robin at 128-node
block granularity to the 8 cores (sorted-rank s -> block g=s//128,
lane=s%128 -> core c=g%8, local block j=g//8, table row = c*PC+j*128+lane).
Per block-rank j the chunk schedule is shared by all cores (SPMD: one
program, per-core tensor data).  Each dst node's edges occupy "slots"; a
chunk is slot k of all 128 lanes of a block.

PAIR GATHER: dma_gather is int16-indexed, and VPAD (50176) exceeds 32768
rows.  Instead of splitting the table into two index regions (which costs
max(Binom(deg,1/2)) twice per block), each descriptor fetches a PAIR of
adjacent table rows: idx = srcrow//2 in [0, VPAD/2) fits int16 with no
split, and the chunk count per block is exactly the block's max degree.
Host-built parity masks (interleaved even/odd per slot) select the needed
row of each pair on-chip: w_par = exp(prelu(asrc_par+adst)) * mask_par and
both parities' weighted rows accumulate in the same chunk-axis reduction.

Attention vectors are folded into the feature matmul (W_aug = [W | W@a_src |
W@a_dst]) and the bias into the gather table (h+b), which adds b exactly
after softmax normalization.

Device, per core (Tile): h-phase matmuls build the gather table
[h+b | alpha_src] and the local alpha_dst column; AllGather replicates the
table; aggregation gathers source-row pairs with multi-chunk dma_gather,
computes w on ACT/DVE, builds V = [w*h | w] on DVE, and segment-sums via
DVE tensor_reduce over the chunk axis accumulated in SBUF; epilogue fuses
out = act(num * (1/den)) on ACT (scale = per-lane reciprocal), with the
layer-2 h-phase inline per block.  The final sigmoid rows are written per
block; the host undoes the row permutation.
"""

import numpy as np

NCORES = 8
F_IN = 128
HID = 64
HEADS = 2
OUT = 64
NEG_SLOPE = 0.2

TROW1 = 192  # L1 table row bf16: [h+b1 (128 bf16) | asrc (2 f32 bits) | pad] (384B)
TROW2 = 128  # L2 table row bf16: [h2+b2 (64 bf16) | asrc2 (1 f32 bits) | pad] (256B)
V1C = 130    # L1 V cols: [w*h (128) | w (2 heads)]
V2C = 65     # L2 V cols: [w*h2 (64) | w]
GBATCH = 22  # max chunks per dma_gather

TRACE = False
_cache = {}


def _build_schedule(edge_index, n_nodes):
    ei = np.asarray(edge_index).astype(np.int64)
    src = np.concatenate([ei[0], np.arange(n_nodes, dtype=np.int64)])
    dst = np.concatenate([ei[1], np.arange(n_nodes, dtype=np.int64)])
    deg = np.bincount(dst, minlength=n_nodes)

    stripe = NCORES * 128
    vpad = ((n_nodes + stripe - 1) // stripe) * stripe
    pc = vpad // NCORES
    nb = pc // 128
    assert vpad // 2 <= 32768  # pair index must fit int16

    degp = np.zeros(vpad, np.int64)
    degp[:n_nodes] = deg
    order = np.argsort(-degp, kind="stable")
    rank = np.empty(vpad, np.int64)
    rank[order] = np.arange(vpad)

    s = np.arange(vpad)
    g = s // 128
    lane = s % 128
    row_of_rank = (g % NCORES) * pc + (g // NCORES) * 128 + lane
    row_of_node = row_of_rank[rank[:n_nodes]]

    e_dstrow = row_of_node[dst]
    e_srcrow = row_of_node[src]

    cnt = np.bincount(e_dstrow, minlength=vpad)
    # per-block-rank shared chunk counts: max over the 8 cores' j-th blocks
    jj = (np.arange(vpad) % pc) // 128
    K = np.zeros(nb, np.int64)
    np.maximum.at(K, jj, cnt)
    K = np.maximum(K, 1)
    nch = int(K.sum())
    chunk_base = np.concatenate([[0], np.cumsum(K)])[:-1]

    # slot assignment: edges of a dst grouped contiguously
    ord_e = np.argsort(e_dstrow, kind="stable")
    ds = e_dstrow[ord_e]
    first = np.r_[True, ds[1:] != ds[:-1]]
    grp_start = np.flatnonzero(first)
    grp_id = np.cumsum(first) - 1
    slot = np.arange(ds.shape[0]) - grp_start[grp_id]
    c = ds // pc
    j = (ds % pc) // 128
    ln = ds % 128
    pos = chunk_base[j] + slot
    assert (slot < K[j]).all()

    sr = e_srcrow[ord_e]
    idx_stream = np.zeros((NCORES, 128, nch), np.int16)
    msk = np.zeros((NCORES, 128, 2 * nch), np.float32)
    # All-padding lanes (zero-degree pad nodes) get a 1e-30 weight on their
    # first slot so the softmax denominator never reaches exactly 0 (their
    # output rows are discarded by the host anyway).
    pad_rows = np.setdiff1d(np.arange(vpad), row_of_node, assume_unique=False)
    pr_c = pad_rows // pc
    pr_j = (pad_rows % pc) // 128
    pr_ln = pad_rows % 128
    msk[pr_c, pr_ln, 2 * chunk_base[pr_j]] = 1e-30
    idx_stream[c, ln, pos] = (sr // 2).astype(np.int16)
    msk[c, ln, 2 * pos + (sr & 1)] = 1.0

    # wrapped int16 layout for dma_gather: chunk k -> columns 8k:8k+8 of
    # [128, 8*nch]; within a chunk the 128 lane-indices are wrapped as
    # flat[i] -> [i % 16, i // 16] and replicated over the 8 16-partition
    # groups.
    iw = idx_stream.transpose(0, 2, 1).reshape(NCORES, nch, 8, 16)
    iw = iw.transpose(0, 3, 1, 2).reshape(NCORES, 16, nch * 8)
    idx_wrapped = np.tile(iw, (1, 8, 1))  # [NCORES, 128, nch*8]

    return dict(vpad=vpad, pc=pc, nb=nb, K=K, nch=nch, chunk_base=chunk_base,
                row_of_node=row_of_node,
                idx_wrapped=np.ascontiguousarray(idx_wrapped),
                mask_stream=msk)


def _build_program(vpad, pc, nb, K, nch, chunk_base):
    import concourse.bacc as bacc
    import concourse.mybir as mybir
    import concourse.tile as tile
    from concourse.masks import make_identity

    F32 = mybir.dt.float32
    BF16 = mybir.dt.bfloat16
    I16 = mybir.dt.int16
    ACTF = mybir.ActivationFunctionType
    ALU = mybir.AluOpType
    AXL = mybir.AxisListType

    nc = bacc.Bacc("TRN2", target_bir_lowering=False, debug=False,
                   num_devices=NCORES, num_swdge_queues=4)

    xt_d = nc.dram_tensor("xt", [128, pc], BF16, kind="ExternalInput")
    idx_d = nc.dram_tensor("idx", [128, nch * 8], I16, kind="ExternalInput")
    msk_d = nc.dram_tensor("msk", [128, 2 * nch], BF16, kind="ExternalInput")
    w1_d = nc.dram_tensor("w1aug", [128, 132], BF16, kind="ExternalInput")
    w2_d = nc.dram_tensor("w2aug", [128, 66], BF16, kind="ExternalInput")
    b1_d = nc.dram_tensor("b1rep", [128, 128], F32, kind="ExternalInput")
    b2_d = nc.dram_tensor("b2rep", [128, 64], F32, kind="ExternalInput")
    out_d = nc.dram_tensor("out", [pc, OUT], F32, kind="ExternalOutput")

    qn = [0]

    with tile.TileContext(nc) as tc:
        with (
            tc.tile_pool(name="const", bufs=1) as cp,
            tc.tile_pool(name="dram", bufs=1, space="DRAM") as dp,
            tc.tile_pool(name="hrow", bufs=3) as hp,
            tc.tile_pool(name="psh", bufs=2, space="PSUM") as psh,
            tc.tile_pool(name="g", bufs=8) as gp,
            tc.tile_pool(name="v", bufs=2) as vp,
            tc.tile_pool(name="wz", bufs=4) as wp,
            tc.tile_pool(name="acc", bufs=2) as ap_,
            tc.tile_pool(name="pst", bufs=1, space="PSUM") as pst,
            tc.tile_pool(name="psa", bufs=3, space="PSUM") as psa,
            tc.tile_pool(name="epi", bufs=3) as ep,
        ):
            ident = cp.tile([128, 128], F32)
            make_identity(nc, ident[:])
            identb = cp.tile([128, 128], BF16)
            nc.scalar.activation(identb[:], ident[:], ACTF.Copy)
            adw1_sb = cp.tile([128, 2 * ((pc // 128))], F32)
            adw2_sb = cp.tile([128, (pc // 128)], F32)
            w1_sb = cp.tile([128, 132], BF16)
            w2_sb = cp.tile([128, 66], BF16)
            b1_sb = cp.tile([128, 128], F32)
            b2_sb = cp.tile([128, 64], F32)
            idx_t = cp.tile([128, nch * 8], I16)
            msk_t = cp.tile([128, 2 * nch], BF16)
            for t, d in ((w1_sb, w1_d), (w2_sb, w2_d), (b1_sb, b1_d),
                         (b2_sb, b2_d), (idx_t, idx_d), (msk_t, msk_d)):
                nc.sync.dma_start(out=t[:], in_=d[:])

            h_loc = dp.tile([pc, TROW1], BF16)
            h_full = dp.tile([vpad, TROW1], BF16, addr_space="Shared")
            h2_loc = dp.tile([pc, TROW2], BF16)
            h2_full = dp.tile([vpad, TROW2], BF16, addr_space="Shared")

            xt_all = cp.tile([128, pc], BF16)
            nc.sync.dma_start(out=xt_all[:], in_=xt_d[:])

            # ---- Phase 1: L1 h-phase ----
            for j in range(nb):
                ps = psh.tile([128, 132], F32, tag="psh")
                nc.tensor.matmul(ps[:], lhsT=xt_all[:, j * 128:(j + 1) * 128],
                                 rhs=w1_sb[:], start=True, stop=True)
                hrow = hp.tile([128, TROW1], BF16, tag="hrow")
                nc.vector.tensor_tensor(out=hrow[:, 0:128], in0=ps[:, 0:128],
                                        in1=b1_sb[:], op=ALU.add)
                # asrc|adst kept as exact f32 bits inside the bf16 row
                nc.scalar.activation(hrow[:, 128:136].bitcast(F32),
                                     ps[:, 128:132], ACTF.Copy)
                nc.scalar.activation(adw1_sb[:, 2 * j:2 * j + 2],
                                     ps[:, 130:132], ACTF.Copy)
                nc.sync.dma_start(out=h_loc[j * 128:(j + 1) * 128, 0:136],
                                  in_=hrow[:, 0:136])

            # ---- Phase 2: AllGather L1 table ----
            nc.gpsimd.collective_compute(
                "AllGather", mybir.AluOpType.bypass,
                replica_groups=[list(range(NCORES))],
                ins=[h_loc[:]], outs=[h_full[:]],
            )

            def agg_layer(layer):
                if layer == 1:
                    table, loc, acol0, trow, vcols, heads = (
                        h_full, h_loc, 132, TROW1, V1C, 2)
                else:
                    table, loc, acol0, trow, vcols, heads = (
                        h2_full, h2_loc, 66, TROW2, V2C, 1)
                hdim = (vcols - heads) // heads
                grow = 2 * trow  # pair row
                tab_pairs = table[:].rearrange("(q t) c -> q (t c)", t=2)
                adw_all = adw1_sb if layer == 1 else adw2_sb
                for j in range(nb):
                    # alternate the segment-sum between the idle tensor
                    # engine (identity-matmul PSUM accumulation) and DVE
                    # (tensor_reduce) so neither gates the gather pipeline
                    use_mm = (j % 2 == 0)
                    if use_mm:
                        acc = psa.tile([128, vcols], F32, tag="psa")
                    else:
                        acc = ap_.tile([128, vcols], F32, tag="acc")
                    kj = int(K[j])
                    cb = int(chunk_base[j])
                    b0 = 0
                    while b0 < kj:
                        gl = min(GBATCH, kj - b0)
                        gl2 = 2 * gl
                        k0 = cb + b0
                        gt = gp.tile([128, GBATCH * grow], BF16, tag="g")
                        nc.gpsimd.dma_gather(
                            gt[:, 0:gl * grow].rearrange("p (k c) -> p k c",
                                                         c=grow),
                            tab_pairs,
                            idx_t[:, k0 * 8:(k0 + gl) * 8],
                            gl * 128, gl * 128, grow,
                            single_packet=False, queue_num=qn[0],
                        )
                        qn[0] = (qn[0] + 1) % 4
                        # [128, gl2, trow]: k2 = chunk*2 + parity
                        gv = gt[:, 0:gl * grow].rearrange("p (k c) -> p k c",
                                                          c=trow)
                        # f32 view of each row (asrc stored as exact f32 bits)
                        gf = gt[:, 0:gl * grow].bitcast(F32).rearrange(
                            "p (k c) -> p k c", c=trow // 2)
                        acol = heads * hdim // 2
                        vt = vp.tile([128, GBATCH * 2 * vcols], BF16, tag="v")
                        vv = vt[:, 0:gl2 * vcols].rearrange(
                            "p (k c) -> p k c", c=vcols)
                        zv = vv[:, :, heads * hdim:vcols]
                        for h in range(heads):
                            nc.scalar.activation(
                                zv[:, :, h], gf[:, :, acol + h],
                                ACTF.Prelu,
                                bias=adw_all[:, j * heads + h:j * heads + h + 1],
                                alpha=NEG_SLOPE)
                            nc.scalar.activation(zv[:, :, h], zv[:, :, h],
                                                 ACTF.Exp)
                        nc.vector.tensor_tensor(
                            out=zv[:, :, :],
                            in0=zv[:, :, :],
                            in1=msk_t[:, 2 * k0:2 * k0 + gl2].to_broadcast(
                                [128, gl2, heads]),
                            op=ALU.mult)
                        for h in range(heads):
                            nc.vector.tensor_tensor(
                                out=vv[:, :, h * hdim:(h + 1) * hdim],
                                in0=gv[:, :, h * hdim:(h + 1) * hdim],
                                in1=zv[:, :, h:h + 1]
                                    .to_broadcast([128, gl2, hdim]),
                                op=ALU.mult)
                        # segment-sum over the chunk axis
                        if use_mm:
                            for k2 in range(gl2):
                                nc.tensor.matmul(
                                    acc[:], lhsT=identb[:],
                                    rhs=vv[:, k2, :],
                                    start=(b0 == 0 and k2 == 0),
                                    stop=(b0 + gl == kj and k2 == gl2 - 1))
                        else:
                            part = wp.tile([128, vcols], F32, tag="part")
                            red_in = vt[:, 0:gl2 * vcols].rearrange(
                                "p (k c) -> p c k", c=vcols)
                            if b0 == 0:
                                nc.vector.tensor_reduce(
                                    acc[:], red_in, axis=AXL.X, op=ALU.add)
                            else:
                                nc.vector.tensor_reduce(
                                    part[:], red_in, axis=AXL.X, op=ALU.add)
                                nc.vector.tensor_tensor(
                                    out=acc[:], in0=acc[:],
                                    in1=part[:], op=ALU.add)
                        b0 += gl

                    # epilogue (den > 0 always: pad lanes carry a 1e-30 seed)
                    rden = wp.tile([128, heads], F32, tag="rden")
                    nc.vector.reciprocal(rden[:], acc[:, heads * hdim:vcols])
                    if layer == 1:
                        h2pre = ep.tile([128, 128], F32, tag="h2pre")
                        for h in range(heads):
                            nc.scalar.activation(
                                h2pre[:, h * hdim:(h + 1) * hdim],
                                acc[:, h * hdim:(h + 1) * hdim],
                                ACTF.Relu, scale=rden[:, h:h + 1])
                        tps = pst.tile([128, 128], F32, tag="tps")
                        nc.tensor.transpose(out=tps[:], in_=h2pre[:],
                                            identity=ident[:])
                        h2t = ep.tile([128, 128], BF16, tag="h2t")
                        nc.scalar.activation(h2t[:], tps[:], ACTF.Copy)
                        ps3 = psh.tile([128, 66], F32, tag="psh")
                        nc.tensor.matmul(ps3[:], lhsT=h2t[:], rhs=w2_sb[:],
                                         start=True, stop=True)
                        h2row = hp.tile([128, TROW2], BF16, tag="h2row")
                        nc.vector.tensor_tensor(out=h2row[:, 0:64],
                                                in0=ps3[:, 0:64], in1=b2_sb[:],
                                                op=ALU.add)
                        nc.scalar.activation(h2row[:, 64:68].bitcast(F32),
                                             ps3[:, 64:66], ACTF.Copy)
                        nc.scalar.activation(adw2_sb[:, j:j + 1],
                                             ps3[:, 65:66], ACTF.Copy)
                        nc.sync.dma_start(
                            out=h2_loc[j * 128:(j + 1) * 128, 0:68],
                            in_=h2row[:, 0:68])
                    else:
                        ob = ep.tile([128, OUT], F32, tag="ob")
                        nc.scalar.activation(ob[:], acc[:, 0:OUT],
                                             ACTF.Sigmoid, scale=rden[:, 0:1])
                        nc.sync.dma_start(out=out_d[j * 128:(j + 1) * 128, :],
                                          in_=ob[:])

            agg_layer(1)
            nc.gpsimd.collective_compute(
                "AllGather", mybir.AluOpType.bypass,
                replica_groups=[list(range(NCORES))],
                ins=[h2_loc[:]], outs=[h2_full[:]],
            )
            agg_layer(2)

    nc.finalize()
    return nc


def kernel(x, edge_index, W1, att_src1, att_dst1, b1, W2, att_src2, att_dst2,
           b2):
    import ml_dtypes
    from concourse import bass_utils

    x = np.asarray(x, np.float32)
    W1 = np.asarray(W1, np.float32)
    W2 = np.asarray(W2, np.float32)
    att_src1 = np.asarray(att_src1, np.float32)
    att_dst1 = np.asarray(att_dst1, np.float32)
    att_src2 = np.asarray(att_src2, np.float32)
    att_dst2 = np.asarray(att_dst2, np.float32)
    b1 = np.asarray(b1, np.float32)
    b2 = np.asarray(b2, np.float32)
    n_nodes = x.shape[0]

    sch = _build_schedule(edge_index, n_nodes)
    vpad, pc = sch["vpad"], sch["pc"]

    W1r = W1.reshape(F_IN, HEADS, HID)
    w1_aug = np.zeros((F_IN, 132), np.float32)
    w1_aug[:, 0:HEADS * HID] = W1
    for h in range(HEADS):
        w1_aug[:, HEADS * HID + h] = W1r[:, h, :] @ att_src1[h]
        w1_aug[:, HEADS * HID + HEADS + h] = W1r[:, h, :] @ att_dst1[h]
    w2_aug = np.zeros((HEADS * HID, 66), np.float32)
    w2_aug[:, 0:OUT] = W2
    w2_aug[:, OUT] = W2 @ att_src2[0]
    w2_aug[:, OUT + 1] = W2 @ att_dst2[0]
    b1_rep = np.broadcast_to(b1, (128, HEADS * HID)).copy()
    b2_rep = np.broadcast_to(b2, (128, OUT)).copy()

    x_rho = np.zeros((vpad, F_IN), np.float32)
    x_rho[sch["row_of_node"]] = x

    key = (vpad, sch["nch"], tuple(sch["K"].tolist()))
    if key not in _cache:
        _cache[key] = _build_program(vpad, pc, sch["nb"], sch["K"],
                                     sch["nch"], sch["chunk_base"])
    nc = _cache[key]

    in_maps = []
    for c in range(NCORES):
        in_maps.append({
            "xt": np.ascontiguousarray(x_rho[c * pc:(c + 1) * pc].T)
                .astype(ml_dtypes.bfloat16),
            "idx": sch["idx_wrapped"][c],
            "msk": sch["mask_stream"][c].astype(ml_dtypes.bfloat16),
            "w1aug": w1_aug.astype(ml_dtypes.bfloat16),
            "w2aug": w2_aug.astype(ml_dtypes.bfloat16),
            "b1rep": b1_rep,
            "b2rep": b2_rep,
        })
    res = bass_utils.run_bass_kernel_spmd(nc, in_maps,
                                          core_ids=list(range(NCORES)),
                                          trace=TRACE)
    kernel.last_exec_ns = res.exec_time_ns
    kernel.last_mean_ns = res.mean_exec_time_ns
    out_all = np.concatenate([res.results[c]["out"] for c in range(NCORES)], 0)
    return out_all[sch["row_of_node"][:n_nodes]]


# revision 68
# speedup vs baseline: 1.0323x; 1.0323x over previous
"""Two-layer GAT on 8 Trainium2 NeuronCores (Bass/Tile).

Host (numpy): append self-loops, degree-sort nodes (desc), pad node count to
VPAD (multiple of 8*128) and assign sorted nodes round-robin at 128-node
block granularity to the 8 cores (sorted-rank s -> block g=s//128,
lane=s%128 -> core c=g%8, local block j=g//8, table row = c*PC+j*128+lane).
Per block-rank j the chunk schedule is shared by all cores (SPMD: one
program, per-core tensor data).  Each dst node's edges occupy "slots"; a
chunk is slot k of all 128 lanes of a block.

PAIR GATHER: dma_gather is int16-indexed, and VPAD (50176) exceeds 32768
rows.  Instead of splitting the table into two index regions (which costs
max(Binom(deg,1/2)) twice per block), each descriptor fetches a PAIR of
adjacent table rows: idx = srcrow//2 in [0, VPAD/2) fits int16 with no
split, and the chunk count per block is exactly the block's max degree.
Host-built parity masks (interleaved even/odd per slot) select the needed
row of each pair on-chip: w_par = exp(prelu(asrc_par+adst)) * mask_par and
both parities' weighted rows accumulate in the same chunk-axis reduction.

Attention vectors are folded into the feature matmul (W_aug = [W | W@a_src |
W@a_dst]) and the bias into the gather table (h+b), which adds b exactly
after softmax normalization.

Device, per core (Tile): h-phase matmuls build the gather table
[h+b | alpha_src] and the local alpha_dst column; AllGather replicates the
table; aggregation gathers source-row pairs with multi-chunk dma_gather,
computes w on ACT/DVE, builds V = [w*h | w] on DVE, and segment-sums via
DVE tensor_reduce over the chunk axis accumulated in SBUF; epilogue fuses
out = act(num * (1/den)) on ACT (scale = per-lane reciprocal), with the
layer-2 h-phase inline per block.  The final sigmoid rows are written per
block; the host undoes the row permutation.
"""

import numpy as np

NCORES = 8
F_IN = 128
HID = 64
HEADS = 2
OUT = 64
NEG_SLOPE = 0.2

TROW1 = 192  # L1 table row bf16: [h+b1 (128 bf16) | asrc (2 f32 bits) | pad] (384B)
TROW2 = 128  # L2 table row bf16: [h2+b2 (64 bf16) | asrc2 (1 f32 bits) | pad] (256B)
V1C = 130    # L1 V cols: [w*h (128) | w (2 heads)]
V2C = 65     # L2 V cols: [w*h2 (64) | w]
GBATCH = 24  # max chunks per dma_gather

TRACE = False
_cache = {}


def _build_schedule(edge_index, n_nodes):
    ei = np.asarray(edge_index).astype(np.int64)
    src = np.concatenate([ei[0], np.arange(n_nodes, dtype=np.int64)])
    dst = np.concatenate([ei[1], np.arange(n_nodes, dtype=np.int64)])
    deg = np.bincount(dst, minlength=n_nodes)

    stripe = NCORES * 128
    vpad = ((n_nodes + stripe - 1) // stripe) * stripe
    pc = vpad // NCORES
    nb = pc // 128
    assert vpad // 2 <= 32768  # pair index must fit int16

    degp = np.zeros(vpad, np.int64)
    degp[:n_nodes] = deg
    order = np.argsort(-degp, kind="stable")
    rank = np.empty(vpad, np.int64)
    rank[order] = np.arange(vpad)

    s = np.arange(vpad)
    g = s // 128
    lane = s % 128
    row_of_rank = (g % NCORES) * pc + (g // NCORES) * 128 + lane
    row_of_node = row_of_rank[rank[:n_nodes]]

    e_dstrow = row_of_node[dst]
    e_srcrow = row_of_node[src]

    cnt = np.bincount(e_dstrow, minlength=vpad)
    # per-block-rank shared chunk counts: max over the 8 cores' j-th blocks
    jj = (np.arange(vpad) % pc) // 128
    K = np.zeros(nb, np.int64)
    np.maximum.at(K, jj, cnt)
    K = np.maximum(K, 1)
    nch = int(K.sum())
    chunk_base = np.concatenate([[0], np.cumsum(K)])[:-1]

    # slot assignment: edges of a dst grouped contiguously
    ord_e = np.argsort(e_dstrow, kind="stable")
    ds = e_dstrow[ord_e]
    first = np.r_[True, ds[1:] != ds[:-1]]
    grp_start = np.flatnonzero(first)
    grp_id = np.cumsum(first) - 1
    slot = np.arange(ds.shape[0]) - grp_start[grp_id]
    c = ds // pc
    j = (ds % pc) // 128
    ln = ds % 128
    pos = chunk_base[j] + slot
    assert (slot < K[j]).all()

    sr = e_srcrow[ord_e]
    idx_stream = np.zeros((NCORES, 128, nch), np.int16)
    msk = np.zeros((NCORES, 128, 2 * nch), np.float32)
    # All-padding lanes (zero-degree pad nodes) get a 1e-30 weight on their
    # first slot so the softmax denominator never reaches exactly 0 (their
    # output rows are discarded by the host anyway).
    pad_rows = np.setdiff1d(np.arange(vpad), row_of_node, assume_unique=False)
    pr_c = pad_rows // pc
    pr_j = (pad_rows % pc) // 128
    pr_ln = pad_rows % 128
    msk[pr_c, pr_ln, 2 * chunk_base[pr_j]] = 1e-30
    idx_stream[c, ln, pos] = (sr // 2).astype(np.int16)
    msk[c, ln, 2 * pos + (sr & 1)] = 1.0

    # wrapped int16 layout for dma_gather: chunk k -> columns 8k:8k+8 of
    # [128, 8*nch]; within a chunk the 128 lane-indices are wrapped as
    # flat[i] -> [i % 16, i // 16] and replicated over the 8 16-partition
    # groups.
    iw = idx_stream.transpose(0, 2, 1).reshape(NCORES, nch, 8, 16)
    iw = iw.transpose(0, 3, 1, 2).reshape(NCORES, 16, nch * 8)
    idx_wrapped = np.tile(iw, (1, 8, 1))  # [NCORES, 128, nch*8]

    return dict(vpad=vpad, pc=pc, nb=nb, K=K, nch=nch, chunk_base=chunk_base,
                row_of_node=row_of_node,
                idx_wrapped=np.ascontiguousarray(idx_wrapped),
                mask_stream=msk)


def _build_program(vpad, pc, nb, K, nch, chunk_base):
    import concourse.bacc as bacc
    import concourse.mybir as mybir
    import concourse.tile as tile
    from concourse.masks import make_identity

    F32 = mybir.dt.float32
    BF16 = mybir.dt.bfloat16
    I16 = mybir.dt.int16
    ACTF = mybir.ActivationFunctionType
    ALU = mybir.AluOpType
    AXL = mybir.AxisListType

    nc = bacc.Bacc("TRN2", target_bir_lowering=False, debug=False,
                   num_devices=NCORES, num_swdge_queues=4)

    xt_d = nc.dram_tensor("xt", [128, pc], BF16, kind="ExternalInput")
    idx_d = nc.dram_tensor("idx", [128, nch * 8], I16, kind="ExternalInput")
    msk_d = nc.dram_tensor("msk", [128, 2 * nch], BF16, kind="ExternalInput")
    w1_d = nc.dram_tensor("w1aug", [128, 132], BF16, kind="ExternalInput")
    w2_d = nc.dram_tensor("w2aug", [128, 66], BF16, kind="ExternalInput")
    b1_d = nc.dram_tensor("b1rep", [128, 128], F32, kind="ExternalInput")
    b2_d = nc.dram_tensor("b2rep", [128, 64], F32, kind="ExternalInput")
    out_d = nc.dram_tensor("out", [pc, OUT], F32, kind="ExternalOutput")

    qn = [0]

    with tile.TileContext(nc) as tc:
        with (
            tc.tile_pool(name="const", bufs=1) as cp,
            tc.tile_pool(name="dram", bufs=1, space="DRAM") as dp,
            tc.tile_pool(name="hrow", bufs=3) as hp,
            tc.tile_pool(name="psh", bufs=2, space="PSUM") as psh,
            tc.tile_pool(name="g", bufs=7) as gp,
            tc.tile_pool(name="v", bufs=3) as vp,
            tc.tile_pool(name="wz", bufs=4) as wp,
            tc.tile_pool(name="acc", bufs=3) as ap_,
            tc.tile_pool(name="pst", bufs=1, space="PSUM") as pst,
            tc.tile_pool(name="psa", bufs=4, space="PSUM") as psa,
            tc.tile_pool(name="epi", bufs=3) as ep,
        ):
            ident = cp.tile([128, 128], F32)
            make_identity(nc, ident[:])
            identb = cp.tile([128, 128], BF16)
            nc.scalar.activation(identb[:], ident[:], ACTF.Copy)
            adw1_sb = cp.tile([128, 2 * ((pc // 128))], F32)
            adw2_sb = cp.tile([128, (pc // 128)], F32)
            w1_sb = cp.tile([128, 132], BF16)
            w2_sb = cp.tile([128, 66], BF16)
            b1_sb = cp.tile([128, 128], F32)
            b2_sb = cp.tile([128, 64], F32)
            idx_t = cp.tile([128, nch * 8], I16)
            msk_t = cp.tile([128, 2 * nch], BF16)
            for t, d in ((w1_sb, w1_d), (w2_sb, w2_d), (b1_sb, b1_d),
                         (b2_sb, b2_d), (idx_t, idx_d), (msk_t, msk_d)):
                nc.sync.dma_start(out=t[:], in_=d[:])

            h_loc = dp.tile([pc, TROW1], BF16)
            h_full = dp.tile([vpad, TROW1], BF16, addr_space="Shared")
            h2_loc = dp.tile([pc, TROW2], BF16)
            h2_full = dp.tile([vpad, TROW2], BF16, addr_space="Shared")

            xt_all = cp.tile([128, pc], BF16)
            nc.sync.dma_start(out=xt_all[:], in_=xt_d[:])

            # ---- Phase 1: L1 h-phase ----
            for j in range(nb):
                ps = psh.tile([128, 132], F32, tag="psh")
                nc.tensor.matmul(ps[:], lhsT=xt_all[:, j * 128:(j + 1) * 128],
                                 rhs=w1_sb[:], start=True, stop=True)
                hrow = hp.tile([128, TROW1], BF16, tag="hrow")
                nc.vector.tensor_tensor(out=hrow[:, 0:128], in0=ps[:, 0:128],
                                        in1=b1_sb[:], op=ALU.add)
                # asrc|adst kept as exact f32 bits inside the bf16 row
                nc.scalar.activation(hrow[:, 128:136].bitcast(F32),
                                     ps[:, 128:132], ACTF.Copy)
                nc.scalar.activation(adw1_sb[:, 2 * j:2 * j + 2],
                                     ps[:, 130:132], ACTF.Copy)
                nc.sync.dma_start(out=h_loc[j * 128:(j + 1) * 128, 0:136],
                                  in_=hrow[:, 0:136])

            # ---- Phase 2: AllGather L1 table ----
            nc.gpsimd.collective_compute(
                "AllGather", mybir.AluOpType.bypass,
                replica_groups=[list(range(NCORES))],
                ins=[h_loc[:]], outs=[h_full[:]],
            )

            def agg_layer(layer):
                if layer == 1:
                    table, loc, acol0, trow, vcols, heads = (
                        h_full, h_loc, 132, TROW1, V1C, 2)
                else:
                    table, loc, acol0, trow, vcols, heads = (
                        h2_full, h2_loc, 66, TROW2, V2C, 1)
                hdim = (vcols - heads) // heads
                grow = 2 * trow  # pair row
                tab_pairs = table[:].rearrange("(q t) c -> q (t c)", t=2)
                adw_all = adw1_sb if layer == 1 else adw2_sb
                for j in range(nb):
                    # alternate the segment-sum between the idle tensor
                    # engine (identity-matmul PSUM accumulation) and DVE
                    # (tensor_reduce) so neither gates the gather pipeline
                    use_mm = (j % 2 == 0)
                    if use_mm:
                        acc = psa.tile([128, vcols], F32, tag="psa")
                    else:
                        acc = ap_.tile([128, vcols], F32, tag="acc")
                    kj = int(K[j])
                    cb = int(chunk_base[j])
                    b0 = 0
                    while b0 < kj:
                        gl = min(GBATCH, kj - b0)
                        gl2 = 2 * gl
                        k0 = cb + b0
                        gt = gp.tile([128, GBATCH * grow], BF16, tag="g")
                        nc.gpsimd.dma_gather(
                            gt[:, 0:gl * grow].rearrange("p (k c) -> p k c",
                                                         c=grow),
                            tab_pairs,
                            idx_t[:, k0 * 8:(k0 + gl) * 8],
                            gl * 128, gl * 128, grow,
                            single_packet=False, queue_num=qn[0],
                        )
                        qn[0] = (qn[0] + 1) % 4
                        # [128, gl2, trow]: k2 = chunk*2 + parity
                        gv = gt[:, 0:gl * grow].rearrange("p (k c) -> p k c",
                                                          c=trow)
                        # f32 view of each row (asrc stored as exact f32 bits)
                        gf = gt[:, 0:gl * grow].bitcast(F32).rearrange(
                            "p (k c) -> p k c", c=trow // 2)
                        acol = heads * hdim // 2
                        vt = vp.tile([128, GBATCH * 2 * vcols], BF16, tag="v")
                        vv = vt[:, 0:gl2 * vcols].rearrange(
                            "p (k c) -> p k c", c=vcols)
                        zv = vv[:, :, heads * hdim:vcols]
                        for h in range(heads):
                            nc.scalar.activation(
                                zv[:, :, h], gf[:, :, acol + h],
                                ACTF.Prelu,
                                bias=adw_all[:, j * heads + h:j * heads + h + 1],
                                alpha=NEG_SLOPE)
                            nc.scalar.activation(zv[:, :, h], zv[:, :, h],
                                                 ACTF.Exp)
                        nc.vector.tensor_tensor(
                            out=zv[:, :, :],
                            in0=zv[:, :, :],
                            in1=msk_t[:, 2 * k0:2 * k0 + gl2].to_broadcast(
                                [128, gl2, heads]),
                            op=ALU.mult)
                        for h in range(heads):
                            nc.vector.tensor_tensor(
                                out=vv[:, :, h * hdim:(h + 1) * hdim],
                                in0=gv[:, :, h * hdim:(h + 1) * hdim],
                                in1=zv[:, :, h:h + 1]
                                    .to_broadcast([128, gl2, hdim]),
                                op=ALU.mult)
                        # segment-sum over the chunk axis
                        if use_mm:
                            for k2 in range(gl2):
                                nc.tensor.matmul(
                                    acc[:], lhsT=identb[:],
                                    rhs=vv[:, k2, :],
                                    start=(b0 == 0 and k2 == 0),
                                    stop=(b0 + gl == kj and k2 == gl2 - 1))
                        else:
                            part = wp.tile([128, vcols], F32, tag="part")
                            red_in = vt[:, 0:gl2 * vcols].rearrange(
                                "p (k c) -> p c k", c=vcols)
                            if b0 == 0:
                                nc.vector.tensor_reduce(
                                    acc[:], red_in, axis=AXL.X, op=ALU.add)
                            else:
                                nc.vector.tensor_reduce(
                                    part[:], red_in, axis=AXL.X, op=ALU.add)
                                nc.vector.tensor_tensor(
                                    out=acc[:], in0=acc[:],
                                    in1=part[:], op=ALU.add)
                        b0 += gl

                    # epilogue (den > 0 always: pad lanes carry a 1e-30 seed)
                    rden = wp.tile([128, heads], F32, tag="rden")
                    nc.vector.reciprocal(rden[:], acc[:, heads * hdim:vcols])
                    if layer == 1:
                        h2pre = ep.tile([128, 128], F32, tag="h2pre")
                        for h in range(heads):
                            nc.scalar.activation(
                                h2pre[:, h * hdim:(h + 1) * hdim],
                                acc[:, h * hdim:(h + 1) * hdim],
                                ACTF.Relu, scale=rden[:, h:h + 1])
                        tps = pst.tile([128, 128], F32, tag="tps")
                        nc.tensor.transpose(out=tps[:], in_=h2pre[:],
                                            identity=ident[:])
                        h2t = ep.tile([128, 128], BF16, tag="h2t")
                        nc.scalar.activation(h2t[:], tps[:], ACTF.Copy)
                        ps3 = psh.tile([128, 66], F32, tag="psh")
                        nc.tensor.matmul(ps3[:], lhsT=h2t[:], rhs=w2_sb[:],
                                         start=True, stop=True)
                        h2row = hp.tile([128, TROW2], BF16, tag="h2row")
                        nc.vector.tensor_tensor(out=h2row[:, 0:64],
                                                in0=ps3[:, 0:64], in1=b2_sb[:],
                                                op=ALU.add)
                        nc.scalar.activation(h2row[:, 64:68].bitcast(F32),
                                             ps3[:, 64:66], ACTF.Copy)
                        nc.scalar.activation(adw2_sb[:, j:j + 1],
                                             ps3[:, 65:66], ACTF.Copy)
                        nc.sync.dma_start(
                            out=h2_loc[j * 128:(j + 1) * 128, 0:68],
                            in_=h2row[:, 0:68])
                    else:
                        ob = ep.tile([128, OUT], F32, tag="ob")
                        nc.scalar.activation(ob[:], acc[:, 0:OUT],
                                             ACTF.Sigmoid, scale=rden[:, 0:1])
                        nc.sync.dma_start(out=out_d[j * 128:(j + 1) * 128, :],
                                          in_=ob[:])

            agg_layer(1)
            nc.gpsimd.collective_compute(
                "AllGather", mybir.AluOpType.bypass,
                replica_groups=[list(range(NCORES))],
                ins=[h2_loc[:]], outs=[h2_full[:]],
            )
            agg_layer(2)

    nc.finalize()
    return nc


def kernel(x, edge_index, W1, att_src1, att_dst1, b1, W2, att_src2, att_dst2,
           b2):
    import ml_dtypes
    from concourse import bass_utils

    x = np.asarray(x, np.float32)
    W1 = np.asarray(W1, np.float32)
    W2 = np.asarray(W2, np.float32)
    att_src1 = np.asarray(att_src1, np.float32)
    att_dst1 = np.asarray(att_dst1, np.float32)
    att_src2 = np.asarray(att_src2, np.float32)
    att_dst2 = np.asarray(att_dst2, np.float32)
    b1 = np.asarray(b1, np.float32)
    b2 = np.asarray(b2, np.float32)
    n_nodes = x.shape[0]

    sch = _build_schedule(edge_index, n_nodes)
    vpad, pc = sch["vpad"], sch["pc"]

    W1r = W1.reshape(F_IN, HEADS, HID)
    w1_aug = np.zeros((F_IN, 132), np.float32)
    w1_aug[:, 0:HEADS * HID] = W1
    for h in range(HEADS):
        w1_aug[:, HEADS * HID + h] = W1r[:, h, :] @ att_src1[h]
        w1_aug[:, HEADS * HID + HEADS + h] = W1r[:, h, :] @ att_dst1[h]
    w2_aug = np.zeros((HEADS * HID, 66), np.float32)
    w2_aug[:, 0:OUT] = W2
    w2_aug[:, OUT] = W2 @ att_src2[0]
    w2_aug[:, OUT + 1] = W2 @ att_dst2[0]
    b1_rep = np.broadcast_to(b1, (128, HEADS * HID)).copy()
    b2_rep = np.broadcast_to(b2, (128, OUT)).copy()

    x_rho = np.zeros((vpad, F_IN), np.float32)
    x_rho[sch["row_of_node"]] = x

    key = (vpad, sch["nch"], tuple(sch["K"].tolist()))
    if key not in _cache:
        _cache[key] = _build_program(vpad, pc, sch["nb"], sch["K"],
                                     sch["nch"], sch["chunk_base"])
    nc = _cache[key]

    in_maps = []
    for c in range(NCORES):
        in_maps.append({
            "xt": np.ascontiguousarray(x_rho[c * pc:(c + 1) * pc].T)
                .astype(ml_dtypes.bfloat16),
            "idx": sch["idx_wrapped"][c],
            "msk": sch["mask_stream"][c].astype(ml_dtypes.bfloat16),
            "w1aug": w1_aug.astype(ml_dtypes.bfloat16),
            "w2aug": w2_aug.astype(ml_dtypes.bfloat16),
            "b1rep": b1_rep,
            "b2rep": b2_rep,
        })
    res = bass_utils.run_bass_kernel_spmd(nc, in_maps,
                                          core_ids=list(range(NCORES)),
                                          trace=TRACE)
    kernel.last_exec_ns = res.exec_time_ns
    kernel.last_mean_ns = res.mean_exec_time_ns
    out_all = np.concatenate([res.results[c]["out"] for c in range(NCORES)], 0)
    return out_all[sch["row_of_node"][:n_nodes]]


# revision 70
# speedup vs baseline: 1.0528x; 1.0199x over previous
"""Two-layer GAT on 8 Trainium2 NeuronCores (Bass/Tile).

Host (numpy): append self-loops, degree-sort nodes (desc), pad node count to
VPAD (multiple of 8*128) and assign sorted nodes round-robin at 128-node
block granularity to the 8 cores (sorted-rank s -> block g=s//128,
lane=s%128 -> core c=g%8, local block j=g//8, table row = c*PC+j*128+lane).
Per block-rank j the chunk schedule is shared by all cores (SPMD: one
program, per-core tensor data).  Each dst node's edges occupy "slots"; a
chunk is slot k of all 128 lanes of a block.

PAIR GATHER: dma_gather is int16-indexed, and VPAD (50176) exceeds 32768
rows.  Instead of splitting the table into two index regions (which costs
max(Binom(deg,1/2)) twice per block), each descriptor fetches a PAIR of
adjacent table rows: idx = srcrow//2 in [0, VPAD/2) fits int16 with no
split, and the chunk count per block is exactly the block's max degree.
Host-built parity masks (interleaved even/odd per slot) select the needed
row of each pair on-chip: w_par = exp(prelu(asrc_par+adst)) * mask_par and
both parities' weighted rows accumulate in the same chunk-axis reduction.

Attention vectors are folded into the feature matmul (W_aug = [W | W@a_src |
W@a_dst]) and the bias into the gather table (h+b), which adds b exactly
after softmax normalization.

Device, per core (Tile): h-phase matmuls build the gather table
[h+b | alpha_src] and the local alpha_dst column; AllGather replicates the
table; aggregation gathers source-row pairs with multi-chunk dma_gather,
computes w on ACT/DVE, builds V = [w*h | w] on DVE, and segment-sums via
DVE tensor_reduce over the chunk axis accumulated in SBUF; epilogue fuses
out = act(num * (1/den)) on ACT (scale = per-lane reciprocal), with the
layer-2 h-phase inline per block.  The final sigmoid rows are written per
block; the host undoes the row permutation.
"""

import numpy as np

NCORES = 8
F_IN = 128
HID = 64
HEADS = 2
OUT = 64
NEG_SLOPE = 0.2

TROW1 = 192  # L1 table row bf16: [h+b1 (128 bf16) | asrc (2 f32 bits) | pad] (384B)
TROW2 = 128  # L2 table row bf16: [h2+b2 (64 bf16) | asrc2 (1 f32 bits) | pad] (256B)
V1C = 130    # L1 V cols: [w*h (128) | w (2 heads)]
V2C = 65     # L2 V cols: [w*h2 (64) | w]
GBATCH = 24  # max chunks per dma_gather

TRACE = False
_cache = {}


def _build_schedule(edge_index, n_nodes):
    ei = np.asarray(edge_index).astype(np.int64)
    src = np.concatenate([ei[0], np.arange(n_nodes, dtype=np.int64)])
    dst = np.concatenate([ei[1], np.arange(n_nodes, dtype=np.int64)])
    deg = np.bincount(dst, minlength=n_nodes)

    stripe = NCORES * 128
    vpad = ((n_nodes + stripe - 1) // stripe) * stripe
    pc = vpad // NCORES
    nb = pc // 128
    assert vpad // 2 <= 32768  # pair index must fit int16

    degp = np.zeros(vpad, np.int64)
    degp[:n_nodes] = deg
    order = np.argsort(-degp, kind="stable")
    rank = np.empty(vpad, np.int64)
    rank[order] = np.arange(vpad)

    s = np.arange(vpad)
    g = s // 128
    lane = s % 128
    row_of_rank = (g % NCORES) * pc + (g // NCORES) * 128 + lane
    row_of_node = row_of_rank[rank[:n_nodes]]

    e_dstrow = row_of_node[dst]
    e_srcrow = row_of_node[src]

    cnt = np.bincount(e_dstrow, minlength=vpad)
    # per-block-rank shared chunk counts: max over the 8 cores' j-th blocks
    jj = (np.arange(vpad) % pc) // 128
    K = np.zeros(nb, np.int64)
    np.maximum.at(K, jj, cnt)
    K = np.maximum(K, 1)
    nch = int(K.sum())
    chunk_base = np.concatenate([[0], np.cumsum(K)])[:-1]

    # slot assignment: edges of a dst grouped contiguously
    ord_e = np.argsort(e_dstrow, kind="stable")
    ds = e_dstrow[ord_e]
    first = np.r_[True, ds[1:] != ds[:-1]]
    grp_start = np.flatnonzero(first)
    grp_id = np.cumsum(first) - 1
    slot = np.arange(ds.shape[0]) - grp_start[grp_id]
    c = ds // pc
    j = (ds % pc) // 128
    ln = ds % 128
    pos = chunk_base[j] + slot
    assert (slot < K[j]).all()

    sr = e_srcrow[ord_e]
    idx_stream = np.zeros((NCORES, 128, nch), np.int16)
    msk = np.zeros((NCORES, 128, 2 * nch), np.float32)
    # All-padding lanes (zero-degree pad nodes) get a 1e-30 weight on their
    # first slot so the softmax denominator never reaches exactly 0 (their
    # output rows are discarded by the host anyway).
    pad_rows = np.setdiff1d(np.arange(vpad), row_of_node, assume_unique=False)
    pr_c = pad_rows // pc
    pr_j = (pad_rows % pc) // 128
    pr_ln = pad_rows % 128
    msk[pr_c, pr_ln, 2 * chunk_base[pr_j]] = 1e-30
    idx_stream[c, ln, pos] = (sr // 2).astype(np.int16)
    msk[c, ln, 2 * pos + (sr & 1)] = 1.0

    # wrapped int16 layout for dma_gather: chunk k -> columns 8k:8k+8 of
    # [128, 8*nch]; within a chunk the 128 lane-indices are wrapped as
    # flat[i] -> [i % 16, i // 16] and replicated over the 8 16-partition
    # groups.
    iw = idx_stream.transpose(0, 2, 1).reshape(NCORES, nch, 8, 16)
    iw = iw.transpose(0, 3, 1, 2).reshape(NCORES, 16, nch * 8)
    idx_wrapped = np.tile(iw, (1, 8, 1))  # [NCORES, 128, nch*8]

    return dict(vpad=vpad, pc=pc, nb=nb, K=K, nch=nch, chunk_base=chunk_base,
                row_of_node=row_of_node,
                idx_wrapped=np.ascontiguousarray(idx_wrapped),
                mask_stream=msk)


def _build_program(vpad, pc, nb, K, nch, chunk_base):
    import concourse.bacc as bacc
    import concourse.mybir as mybir
    import concourse.tile as tile
    from concourse.masks import make_identity

    F32 = mybir.dt.float32
    BF16 = mybir.dt.bfloat16
    I16 = mybir.dt.int16
    ACTF = mybir.ActivationFunctionType
    ALU = mybir.AluOpType
    AXL = mybir.AxisListType

    nc = bacc.Bacc("TRN2", target_bir_lowering=False, debug=False,
                   num_devices=NCORES, num_swdge_queues=4)

    xt_d = nc.dram_tensor("xt", [128, pc], BF16, kind="ExternalInput")
    idx_d = nc.dram_tensor("idx", [128, nch * 8], I16, kind="ExternalInput")
    msk_d = nc.dram_tensor("msk", [128, 2 * nch], BF16, kind="ExternalInput")
    w1_d = nc.dram_tensor("w1aug", [128, 132], BF16, kind="ExternalInput")
    w2_d = nc.dram_tensor("w2aug", [128, 66], BF16, kind="ExternalInput")
    b1_d = nc.dram_tensor("b1rep", [128, 128], F32, kind="ExternalInput")
    b2_d = nc.dram_tensor("b2rep", [128, 64], F32, kind="ExternalInput")
    out_d = nc.dram_tensor("out", [pc, OUT], F32, kind="ExternalOutput")

    qn = [0]

    with tile.TileContext(nc) as tc:
        with (
            tc.tile_pool(name="const", bufs=1) as cp,
            tc.tile_pool(name="dram", bufs=1, space="DRAM") as dp,
            tc.tile_pool(name="hrow", bufs=3) as hp,
            tc.tile_pool(name="psh", bufs=2, space="PSUM") as psh,
            tc.tile_pool(name="g", bufs=7) as gp,
            tc.tile_pool(name="v", bufs=3) as vp,
            tc.tile_pool(name="wz", bufs=4) as wp,
            tc.tile_pool(name="acc", bufs=3) as ap_,
            tc.tile_pool(name="pst", bufs=1, space="PSUM") as pst,
            tc.tile_pool(name="psa", bufs=4, space="PSUM") as psa,
            tc.tile_pool(name="epi", bufs=3) as ep,
        ):
            ident = cp.tile([128, 128], F32)
            make_identity(nc, ident[:])
            identb = cp.tile([128, 128], BF16)
            nc.scalar.activation(identb[:], ident[:], ACTF.Copy)
            adw1_sb = cp.tile([128, 2 * ((pc // 128))], F32)
            adw2_sb = cp.tile([128, (pc // 128)], F32)
            w1_sb = cp.tile([128, 132], BF16)
            w2_sb = cp.tile([128, 66], BF16)
            b1_sb = cp.tile([128, 128], F32)
            b2_sb = cp.tile([128, 64], F32)
            idx_t = cp.tile([128, nch * 8], I16)
            msk_t = cp.tile([128, 2 * nch], BF16)
            for t, d in ((w1_sb, w1_d), (w2_sb, w2_d), (b1_sb, b1_d),
                         (b2_sb, b2_d), (idx_t, idx_d), (msk_t, msk_d)):
                nc.sync.dma_start(out=t[:], in_=d[:])

            h_loc = dp.tile([pc, TROW1], BF16)
            h_full = dp.tile([vpad, TROW1], BF16, addr_space="Shared")
            h2_loc = dp.tile([pc, TROW2], BF16)
            h2_full = dp.tile([vpad, TROW2], BF16, addr_space="Shared")

            xt_all = cp.tile([128, pc], BF16)
            nc.sync.dma_start(out=xt_all[:], in_=xt_d[:])

            # ---- Phase 1: L1 h-phase ----
            for j in range(nb):
                ps = psh.tile([128, 132], F32, tag="psh")
                nc.tensor.matmul(ps[:], lhsT=xt_all[:, j * 128:(j + 1) * 128],
                                 rhs=w1_sb[:], start=True, stop=True)
                hrow = hp.tile([128, TROW1], BF16, tag="hrow")
                nc.vector.tensor_tensor(out=hrow[:, 0:128], in0=ps[:, 0:128],
                                        in1=b1_sb[:], op=ALU.add)
                # asrc|adst kept as exact f32 bits inside the bf16 row
                nc.scalar.activation(hrow[:, 128:136].bitcast(F32),
                                     ps[:, 128:132], ACTF.Copy)
                nc.scalar.activation(adw1_sb[:, 2 * j:2 * j + 2],
                                     ps[:, 130:132], ACTF.Copy)
                nc.sync.dma_start(out=h_loc[j * 128:(j + 1) * 128, 0:136],
                                  in_=hrow[:, 0:136])

            # ---- Phase 2: AllGather L1 table ----
            nc.gpsimd.collective_compute(
                "AllGather", mybir.AluOpType.bypass,
                replica_groups=[list(range(NCORES))],
                ins=[h_loc[:]], outs=[h_full[:]],
            )

            def agg_layer(layer):
                if layer == 1:
                    table, loc, acol0, trow, vcols, heads = (
                        h_full, h_loc, 132, TROW1, V1C, 2)
                else:
                    table, loc, acol0, trow, vcols, heads = (
                        h2_full, h2_loc, 66, TROW2, V2C, 1)
                hdim = (vcols - heads) // heads
                grow = 2 * trow  # pair row
                tab_pairs = table[:].rearrange("(q t) c -> q (t c)", t=2)
                adw_all = adw1_sb if layer == 1 else adw2_sb
                for j in range(nb):
                    # alternate the segment-sum between the idle tensor
                    # engine (identity-matmul PSUM accumulation) and DVE
                    # (tensor_reduce) so neither gates the gather pipeline
                    use_mm = (j % 2 == 0)
                    if use_mm:
                        acc = psa.tile([128, vcols], F32, tag="psa")
                    else:
                        acc = ap_.tile([128, vcols], F32, tag="acc")
                    kj = int(K[j])
                    cb = int(chunk_base[j])
                    b0 = 0
                    while b0 < kj:
                        gl = min(GBATCH, kj - b0)
                        gl2 = 2 * gl
                        k0 = cb + b0
                        gt = gp.tile([128, GBATCH * grow], BF16, tag="g")
                        nc.gpsimd.dma_gather(
                            gt[:, 0:gl * grow].rearrange("p (k c) -> p k c",
                                                         c=grow),
                            tab_pairs,
                            idx_t[:, k0 * 8:(k0 + gl) * 8],
                            gl * 128, gl * 128, grow,
                            single_packet=False, queue_num=qn[0],
                        )
                        qn[0] = (qn[0] + 1) % 4
                        # [128, gl2, trow]: k2 = chunk*2 + parity
                        gv = gt[:, 0:gl * grow].rearrange("p (k c) -> p k c",
                                                          c=trow)
                        # f32 view of each row (asrc stored as exact f32 bits)
                        gf = gt[:, 0:gl * grow].bitcast(F32).rearrange(
                            "p (k c) -> p k c", c=trow // 2)
                        acol = heads * hdim // 2
                        vt = vp.tile([128, GBATCH * 2 * vcols], BF16, tag="v")
                        vv = vt[:, 0:gl2 * vcols].rearrange(
                            "p (k c) -> p k c", c=vcols)
                        zv = vv[:, :, heads * hdim:vcols]
                        for h in range(heads):
                            nc.scalar.activation(
                                zv[:, :, h], gf[:, :, acol + h],
                                ACTF.Prelu,
                                bias=adw_all[:, j * heads + h:j * heads + h + 1],
                                alpha=NEG_SLOPE)
                            nc.scalar.activation(zv[:, :, h], zv[:, :, h],
                                                 ACTF.Exp)
                        nc.vector.tensor_tensor(
                            out=zv[:, :, :],
                            in0=zv[:, :, :],
                            in1=msk_t[:, 2 * k0:2 * k0 + gl2].to_broadcast(
                                [128, gl2, heads]),
                            op=ALU.mult)
                        for h in range(heads):
                            nc.vector.tensor_tensor(
                                out=vv[:, :, h * hdim:(h + 1) * hdim],
                                in0=gv[:, :, h * hdim:(h + 1) * hdim],
                                in1=zv[:, :, h:h + 1]
                                    .to_broadcast([128, gl2, hdim]),
                                op=ALU.mult)
                        # segment-sum over the chunk axis
                        if use_mm:
                            for k2 in range(gl2):
                                nc.tensor.matmul(
                                    acc[:], lhsT=identb[:],
                                    rhs=vv[:, k2, :],
                                    start=(b0 == 0 and k2 == 0),
                                    stop=(b0 + gl == kj and k2 == gl2 - 1))
                        else:
                            part = wp.tile([128, vcols], F32, tag="part")
                            red_in = vt[:, 0:gl2 * vcols].rearrange(
                                "p (k c) -> p c k", c=vcols)
                            if b0 == 0:
                                nc.vector.tensor_reduce(
                                    acc[:], red_in, axis=AXL.X, op=ALU.add)
                            else:
                                nc.vector.tensor_reduce(
                                    part[:], red_in, axis=AXL.X, op=ALU.add)
                                nc.vector.tensor_tensor(
                                    out=acc[:], in0=acc[:],
                                    in1=part[:], op=ALU.add)
                        b0 += gl

                    # epilogue (den > 0 always: pad lanes carry a 1e-30 seed)
                    rden = wp.tile([128, heads], F32, tag="rden")
                    nc.vector.reciprocal(rden[:], acc[:, heads * hdim:vcols])
                    if layer == 1:
                        h2pre = ep.tile([128, 128], F32, tag="h2pre")
                        for h in range(heads):
                            nc.scalar.activation(
                                h2pre[:, h * hdim:(h + 1) * hdim],
                                acc[:, h * hdim:(h + 1) * hdim],
                                ACTF.Relu, scale=rden[:, h:h + 1])
                        tps = pst.tile([128, 128], F32, tag="tps")
                        nc.tensor.transpose(out=tps[:], in_=h2pre[:],
                                            identity=ident[:])
                        h2t = ep.tile([128, 128], BF16, tag="h2t")
                        nc.scalar.activation(h2t[:], tps[:], ACTF.Copy)
                        ps3 = psh.tile([128, 66], F32, tag="psh")
                        nc.tensor.matmul(ps3[:], lhsT=h2t[:], rhs=w2_sb[:],
                                         start=True, stop=True)
                        h2row = hp.tile([128, TROW2], BF16, tag="h2row")
                        nc.vector.tensor_tensor(out=h2row[:, 0:64],
                                                in0=ps3[:, 0:64], in1=b2_sb[:],
                                                op=ALU.add)
                        nc.scalar.activation(h2row[:, 64:68].bitcast(F32),
                                             ps3[:, 64:66], ACTF.Copy)
                        nc.scalar.activation(adw2_sb[:, j:j + 1],
                                             ps3[:, 65:66], ACTF.Copy)
                        nc.sync.dma_start(
                            out=h2_loc[j * 128:(j + 1) * 128, 0:68],
                            in_=h2row[:, 0:68])
                    else:
                        ob = ep.tile([128, OUT], F32, tag="ob")
                        nc.scalar.activation(ob[:], acc[:, 0:OUT],
                                             ACTF.Sigmoid, scale=rden[:, 0:1])
                        nc.sync.dma_start(out=out_d[j * 128:(j + 1) * 128, :],
                                          in_=ob[:])

            agg_layer(1)
            nc.gpsimd.collective_compute(
                "AllGather", mybir.AluOpType.bypass,
                replica_groups=[list(range(NCORES))],
                ins=[h2_loc[:]], outs=[h2_full[:]],
            )
            agg_layer(2)

    nc.finalize()
    return nc


def kernel(x, edge_index, W1, att_src1, att_dst1, b1, W2, att_src2, att_dst2,
           b2):
    import ml_dtypes
    from concourse import bass_utils

    x = np.asarray(x, np.float32)
    W1 = np.asarray(W1, np.float32)
    W2 = np.asarray(W2, np.float32)
    att_src1 = np.asarray(att_src1, np.float32)
    att_dst1 = np.asarray(att_dst1, np.float32)
    att_src2 = np.asarray(att_src2, np.float32)
    att_dst2 = np.asarray(att_dst2, np.float32)
    b1 = np.asarray(b1, np.float32)
    b2 = np.asarray(b2, np.float32)
    n_nodes = x.shape[0]

    sch = _build_schedule(edge_index, n_nodes)
    vpad, pc = sch["vpad"], sch["pc"]

    W1r = W1.reshape(F_IN, HEADS, HID)
    w1_aug = np.zeros((F_IN, 132), np.float32)
    w1_aug[:, 0:HEADS * HID] = W1
    for h in range(HEADS):
        w1_aug[:, HEADS * HID + h] = W1r[:, h, :] @ att_src1[h]
        w1_aug[:, HEADS * HID + HEADS + h] = W1r[:, h, :] @ att_dst1[h]
    w2_aug = np.zeros((HEADS * HID, 66), np.float32)
    w2_aug[:, 0:OUT] = W2
    w2_aug[:, OUT] = W2 @ att_src2[0]
    w2_aug[:, OUT + 1] = W2 @ att_dst2[0]
    b1_rep = np.broadcast_to(b1, (128, HEADS * HID)).copy()
    b2_rep = np.broadcast_to(b2, (128, OUT)).copy()

    x_rho = np.zeros((vpad, F_IN), np.float32)
    x_rho[sch["row_of_node"]] = x

    key = (vpad, sch["nch"], tuple(sch["K"].tolist()))
    if key not in _cache:
        _cache[key] = _build_program(vpad, pc, sch["nb"], sch["K"],
                                     sch["nch"], sch["chunk_base"])
    nc = _cache[key]

    in_maps = []
    for c in range(NCORES):
        in_maps.append({
            "xt": np.ascontiguousarray(x_rho[c * pc:(c + 1) * pc].T)
                .astype(ml_dtypes.bfloat16),
            "idx": sch["idx_wrapped"][c],
            "msk": sch["mask_stream"][c].astype(ml_dtypes.bfloat16),
            "w1aug": w1_aug.astype(ml_dtypes.bfloat16),
            "w2aug": w2_aug.astype(ml_dtypes.bfloat16),
            "b1rep": b1_rep,
            "b2rep": b2_rep,
        })
    res = bass_utils.run_bass_kernel_spmd(nc, in_maps,
                                          core_ids=list(range(NCORES)),
                                          trace=TRACE)
    kernel.last_exec_ns = res.exec_time_ns
    kernel.last_mean_ns = res.mean_exec_time_ns
    out_all = np.concatenate([res.results[c]["out"] for c in range(NCORES)], 0)
    return out_all[sch["row_of_node"][:n_nodes]]
